# revision 12
# baseline (speedup 1.0000x reference)
"""Multi-head attention (B=2, S=2048, D=1024, H=16, dk=64) on 8 TRN2 NeuronCores.

Sharding: core c -> (batch b = c//4, head-group g = c%4 covering heads
[4g, 4g+4) == feature columns [256g, 256g+256)).  Each core:
  - projects K^T/Q^T (features on partitions) and V (natural layout) for its
    4 heads from the batch's activations (host-pre-transposed),
  - runs softmax attention per head without max-subtraction (scores ~ N(0,1));
    softmax denominators come from an appended ones-column on V,
  - computes the transposed partial output  outT = (Wo_cols.T @ ctx^T)  [1024, 2048].
Host gathers: out[b] = sum_g outT[b,g].T + bo.

All matmul operands use fp32r (TF32-like, 11-bit mantissa) at full PE rate;
accumulation is fp32 in PSUM.  End-to-end rel-err vs fp32 reference ~1e-4.

Schedule: K-proj (streamed) -> Q-proj qp0 -> V-proj (resident) -> Q-proj qp1
-> attention qp0 -> attention qp1 (with qp0's output projection interleaved)
-> output projection qp1.  One PSUM pool with two 2-bank tags shared by all
phases so no pool-boundary serialization; attention overlaps late projections.
"""
import sys

for _p in ("/opt/trn_rl_repo", "/root/.axon_site/_ro/trn_rl_repo"):
    if _p not in sys.path:
        sys.path.append(_p)

import numpy as np

B, S, D, H, DK = 2, 2048, 1024, 16, 64
NCORES = 8
GROUPS = 4            # head-groups per batch (cores per batch)
HG = H // GROUPS      # heads per core = 4
F = HG * DK           # features per core = 256
EB = D // 128         # 8 e-blocks
KC = S // 128         # 16 k-chunks
QP = S // 1024        # 2 q-pair-chunks of 1024


def _round_f32r(x: np.ndarray) -> np.ndarray:
    """Round f32 to the fp32r grid (11-bit mantissa): RNE at bit 12."""
    u = np.ascontiguousarray(x, dtype=np.float32).view(np.uint32)
    low = u & np.uint32(0xFFF)
    half = np.uint32(0x800)
    lsb = (u >> np.uint32(12)) & np.uint32(1)
    round_up = (low > half) | ((low == half) & (lsb == 1))
    u = (u & np.uint32(0xFFFFF000)) + (round_up.astype(np.uint32) << np.uint32(12))
    return u.view(np.float32)


def build_nc(reps: int = 1):
    import concourse.bass as bass
    import concourse.tile as tile
    from concourse import bacc, mybir

    F32 = mybir.dt.float32
    F32R = mybir.dt.float32r
    EXP = mybir.ActivationFunctionType.Exp
    MULT = mybir.AluOpType.mult

    nc = bacc.Bacc("TRN2", target_bir_lowering=False, debug=False,
                   num_devices=NCORES)

    xq = nc.dram_tensor("xq", [D, S], F32R, kind="ExternalInput").ap()
    xk = nc.dram_tensor("xk", [D, S], F32R, kind="ExternalInput").ap()
    xv = nc.dram_tensor("xv", [D, S], F32R, kind="ExternalInput").ap()
    wq = nc.dram_tensor("wq", [D, F], F32R, kind="ExternalInput").ap()
    wk = nc.dram_tensor("wk", [D, F], F32R, kind="ExternalInput").ap()
    wv = nc.dram_tensor("wv", [D, F], F32R, kind="ExternalInput").ap()
    wo = nc.dram_tensor("wo", [F, D], F32R, kind="ExternalInput").ap()
    outT = nc.dram_tensor("outT", [D, S], F32, kind="ExternalOutput").ap()
    # scratch for the softmax-denominator partition-broadcast bounce

    with tile.TileContext(nc) as tc:
        with (
            tc.tile_pool(name="persist", bufs=1) as pp,
            tc.tile_pool(name="woP", bufs=1) as wop,
            tc.tile_pool(name="xs", bufs=2) as xs,
            tc.tile_pool(name="xvp", bufs=1) as xvp,
            tc.tile_pool(name="ws", bufs=2) as ws,
            tc.tile_pool(name="pt", bufs=3) as ptp,
            tc.tile_pool(name="sm", bufs=2) as smp,
            tc.tile_pool(name="ob", bufs=2) as obp,
            tc.tile_pool(name="ps", bufs=2, space="PSUM") as psp,
        ):
            KT = pp.tile([128, 2, S], F32R, tag="kt")    # heads 2fb,2fb+1 on rows
            QT = pp.tile([128, 2, S], F32R, tag="qt")
            Vp = pp.tile([128, KC, HG * (DK + 1)], F32R, tag="vp")  # V'+ones per kc
            CT = pp.tile([128, 2, S], F32R, tag="ct")    # normalized ctx^T
            WO = wop.tile([128, 2, D], F32R, tag="wo")
            ones = nc.const_aps.scalar_like(1.0, Vp[:])
            ones_row = wop.tile([1, 64], F32R, tag="onesrow")
            nc.vector.tensor_copy(
                ones_row[:], nc.const_aps.tensor(1.0, (1, 1), F32)
                .to_broadcast((1, 64)))

            def psum(tag, name):
                # two 2-bank tags, bufs=2 each -> 8 banks total
                return psp.tile([128, 1024], F32, tag=tag, name=name)

            def proj_qT(w_sb, x_t, ps_pair, eb):
                for fb in range(2):
                    for qh in range(2):
                        nc.tensor.matmul(
                            ps_pair[fb][:, qh * 512:(qh + 1) * 512],
                            w_sb[:, eb, fb * 128:(fb + 1) * 128],
                            x_t[:, qh * 512:(qh + 1) * 512],
                            start=(eb == 0), stop=(eb == EB - 1))

            for _rep in range(reps):
                # ---------------- V natural (streamed-in resident xv) ----------
                wv_sb = ws.tile([128, EB, F], F32R, tag="w", name="wv_sb")
                nc.sync.dma_start(wv_sb[:], wv.rearrange("(c p) f -> p c f", p=128))
                xv_sb = xvp.tile([128, EB, S], F32R, tag="xv")
                for eb in range(EB):
                    nc.sync.dma_start(xv_sb[:, eb, :], xv[eb * 128:(eb + 1) * 128, :])
                for rnd in range(2):
                    kcs = list(range(rnd * 8, rnd * 8 + 8))
                    vps = {}
                    for j in range(4):  # tile j holds kc pair (2j, 2j+1) of round
                        t = psum(("kq", "ctx")[j % 2], f"vps{rnd}_{j}")
                        vps[kcs[2 * j]] = t[:, 0:256]        # bank 0
                        vps[kcs[2 * j + 1]] = t[:, 512:768]  # bank 1
                    for eb in range(EB):
                        for kc in kcs:
                            nc.tensor.matmul(
                                vps[kc],
                                xv_sb[:, eb, kc * 128:(kc + 1) * 128],
                                wv_sb[:, eb, :],
                                start=(eb == 0), stop=(eb == EB - 1))
                    for kc in kcs:
                        for h in range(HG):
                            nc.vector.tensor_copy(
                                Vp[:, kc, h * 65:h * 65 + 64],
                                vps[kc][:, h * 64:(h + 1) * 64])
                        nc.vector.tensor_copy(
                            Vp[:, kc, 64::65], ones.to_broadcast((128, HG)))

                # ---------------- K^T (streamed xk) ----------------
                wk_sb = ws.tile([128, EB, F], F32R, tag="w", name="wk_sb")
                nc.sync.dma_start(wk_sb[:], wk.rearrange("(c p) f -> p c f", p=128))
                kps = {(fb, qp): psum(("kq", "ctx")[fb], f"kps{fb}{qp}")
                       for fb in range(2) for qp in range(QP)}
                for eb in range(EB):
                    x_t = xs.tile([128, S], F32R, tag="x", name=f"xk_t{eb}")
                    nc.sync.dma_start(x_t[:], xk[eb * 128:(eb + 1) * 128, :])
                    for qp in range(QP):
                        proj_qT(wk_sb, x_t[:, qp * 1024:(qp + 1) * 1024],
                                [kps[0, qp], kps[1, qp]], eb)
                for fb in range(2):
                    for qp in range(QP):
                        nc.vector.tensor_copy(
                            KT[:, fb, qp * 1024:(qp + 1) * 1024], kps[fb, qp][:])

                # ---------------- Q^T qp0 (streamed half tiles) ----------------
                wq_sb = ws.tile([128, EB, F], F32R, tag="w", name="wq_sb")
                nc.sync.dma_start(wq_sb[:], wq.rearrange("(c p) f -> p c f", p=128))
                qps0 = [psum(("kq", "ctx")[i], f"qps0_{i}") for i in range(2)]
                for eb in range(EB):
                    x_t = xs.tile([128, 1024], F32R, tag="x", name=f"xq0_t{eb}")
                    nc.sync.dma_start(x_t[:], xq[eb * 128:(eb + 1) * 128, 0:1024])
                    proj_qT(wq_sb, x_t, qps0, eb)
                for fb in range(2):
                    nc.vector.tensor_copy(QT[:, fb, 0:1024], qps0[fb][:])

                def proj_q1():
                    qps1 = [psum(("kq", "ctx")[i], f"qps1_{i}") for i in range(2)]
                    for eb in range(EB):
                        x_t = xs.tile([128, 1024], F32R, tag="x", name=f"xq1_t{eb}")
                        nc.sync.dma_start(x_t[:],
                                          xq[eb * 128:(eb + 1) * 128, 1024:2048])
                        proj_qT(wq_sb, x_t, qps1, eb)
                    for fb in range(2):
                        nc.vector.tensor_copy(QT[:, fb, 1024:2048], qps1[fb][:])

                # ============== attention + output projection ==============
                def attention_head(qp, h):
                    fb, ro = divmod(h, 2)
                    ro *= 64
                    qsl = slice(qp * 1024, (qp + 1) * 1024)
                    ctx_ps = psp.tile([65, 1024], F32, tag="ctx",
                                      name=f"ctx{qp}{h}")
                    for kc in range(KC):
                        s_ps = psp.tile([128, 1024], F32, tag="kq",
                                        name=f"s{qp}{h}{kc}")
                        for qh in range(2):
                            nc.tensor.matmul(
                                s_ps[:, qh * 512:(qh + 1) * 512],
                                KT[ro:ro + 64, fb, kc * 128:(kc + 1) * 128],
                                QT[ro:ro + 64, fb, qp * 1024 + qh * 512:][:, :512],
                                start=True, stop=True)
                        p_t = ptp.tile([128, 1024], F32R, tag="p", name="p_t")
                        nc.scalar.activation(p_t[:], s_ps[:], EXP,
                                             scale=1.0 / np.sqrt(DK))
                        for qh in range(2):
                            nc.tensor.matmul(
                                ctx_ps[:, qh * 512:(qh + 1) * 512],
                                Vp[:, kc, h * 65:(h + 1) * 65],
                                p_t[:, qh * 512:(qh + 1) * 512],
                                start=(kc == 0), stop=(kc == KC - 1))
                    # normalize rows 0..63 by row 64 (softmax denominators)
                    rc = smp.tile([1, 1024], F32R, tag="rc", name="rc")
                    with nc.allow_low_precision(reason="f32r recip bcast"):
                        nc.vector.reciprocal(rc[:], ctx_ps[64:65, :])
                    bc_ps = psp.tile([64, 1024], F32, tag="ctx",
                                     name=f"bc{qp}{h}")
                    for qh in range(2):
                        nc.tensor.matmul(
                            bc_ps[:, qh * 512:(qh + 1) * 512], ones_row[:],
                            rc[:, qh * 512:(qh + 1) * 512],
                            start=True, stop=True)
                    bc = smp.tile([64, 1024], F32, tag="bc", name="bc")
                    nc.vector.tensor_copy(bc[:], bc_ps[:])
                    nc.vector.tensor_tensor(
                        CT[ro:ro + 64, fb, qsl], ctx_ps[0:64, :], bc[:], MULT)

                def outproj(qp, ods):
                    qsl = slice(qp * 1024, (qp + 1) * 1024)
                    for od in ods:
                        o_ps = psp.tile([128, 1024], F32, tag="kq",
                                        name=f"o{qp}{od}")
                        for qh in range(2):
                            for fb in range(2):
                                nc.tensor.matmul(
                                    o_ps[:, qh * 512:(qh + 1) * 512],
                                    WO[:, fb, od * 128:(od + 1) * 128],
                                    CT[:, fb, qp * 1024 + qh * 512:][:, :512],
                                    start=(fb == 0), stop=(fb == 1))
                        o_sb = obp.tile([128, 1024], F32, tag="ob", name="o_sb")
                        nc.vector.tensor_copy(o_sb[:], o_ps[:])
                        nc.sync.dma_start(outT[od * 128:(od + 1) * 128, qsl],
                                          o_sb[:])

                attention_head(0, 0)
                proj_q1()   # Q^T qp1 overlaps attention of (qp0, h0)
                nc.sync.dma_start(WO[:], wo.rearrange("(c p) o -> p c o", p=128))
                for h in range(1, HG):
                    attention_head(0, h)
                # interleave qp0's out-proj with qp1's attention
                attention_head(1, 0)
                outproj(0, range(0, 4))
                attention_head(1, 1)
                outproj(0, range(4, 8))
                for h in (2, 3):
                    attention_head(1, h)
                outproj(1, range(8))
    nc.compile()
    nc.remove_dead_allocations()
    return nc


_CACHE = {}


def _get_nc(reps: int = 1):
    if reps not in _CACHE:
        _CACHE[reps] = build_nc(reps)
    return _CACHE[reps]


def make_in_maps(key, query, value, Wq, Wk, Wv, Wo):
    """Build the 8 per-core input maps (host-side shard + transpose + round)."""
    in_maps = []
    xqT = [_round_f32r(np.ascontiguousarray(query[b].T)) for b in range(B)]
    xkT = [_round_f32r(np.ascontiguousarray(key[b].T)) for b in range(B)]
    xvT = [_round_f32r(np.ascontiguousarray(value[b].T)) for b in range(B)]
    for c in range(NCORES):
        b, g = divmod(c, GROUPS)
        fs = slice(g * F, (g + 1) * F)
        in_maps.append({
            "xq": xqT[b],
            "xk": xkT[b],
            "xv": xvT[b],
            "wq": _round_f32r(np.ascontiguousarray(Wq[fs, :].T)),
            "wk": _round_f32r(np.ascontiguousarray(Wk[fs, :].T)),
            "wv": _round_f32r(np.ascontiguousarray(Wv[fs, :].T)),
            "wo": _round_f32r(np.ascontiguousarray(Wo[:, fs].T)),
        })
    return in_maps


def gather_output(results, bo):
    """results: list of 8 dicts with 'outT' [D, S] partials."""
    out = np.empty((B, S, D), dtype=np.float32)
    for b in range(B):
        acc = np.zeros((D, S), dtype=np.float64)
        for g in range(GROUPS):
            acc += results[b * GROUPS + g]["outT"]
        out[b] = acc.T + bo.astype(np.float64)
    return out


def kernel(key, query, value, Wq, Wk, Wv, Wo, bo):
    from concourse.bass_utils import run_bass_kernel_spmd

    key = np.asarray(key, dtype=np.float32)
    query = np.asarray(query, dtype=np.float32)
    value = np.asarray(value, dtype=np.float32)
    Wq = np.asarray(Wq, dtype=np.float32)
    Wk = np.asarray(Wk, dtype=np.float32)
    Wv = np.asarray(Wv, dtype=np.float32)
    Wo = np.asarray(Wo, dtype=np.float32)
    bo = np.asarray(bo, dtype=np.float32)

    nc = _get_nc()
    in_maps = make_in_maps(key, query, value, Wq, Wk, Wv, Wo)
    res = run_bass_kernel_spmd(nc, in_maps, core_ids=list(range(NCORES)))
    return gather_output(res.results, bo)


# revision 18
# speedup vs baseline: 1.0437x; 1.0437x over previous
"""Multi-head attention (B=2, S=2048, D=1024, H=16, dk=64) on 8 TRN2 NeuronCores.

Sharding: core c -> (batch b = c//4, head-group g = c%4 covering heads
[4g, 4g+4) == feature columns [256g, 256g+256)).  Each core:
  - projects K^T/Q^T (features on partitions) and V (natural layout) for its
    4 heads from the batch's activations (host-pre-transposed),
  - runs softmax attention per head without max-subtraction (scores ~ N(0,1));
    softmax denominators come from an appended ones-column on V,
  - computes the transposed partial output  outT = (Wo_cols.T @ ctx^T)  [1024, 2048].
Host gathers: out[b] = sum_g outT[b,g].T + bo.

All matmul operands use fp32r (TF32-like, 11-bit mantissa) at full PE rate;
accumulation is fp32 in PSUM.  End-to-end rel-err vs fp32 reference ~1e-4.

Schedule: K-proj (streamed) -> Q-proj qp0 -> V-proj (resident) -> Q-proj qp1
-> attention qp0 -> attention qp1 (with qp0's output projection interleaved)
-> output projection qp1.  One PSUM pool with two 2-bank tags shared by all
phases so no pool-boundary serialization; attention overlaps late projections.
"""
import sys

for _p in ("/opt/trn_rl_repo", "/root/.axon_site/_ro/trn_rl_repo"):
    if _p not in sys.path:
        sys.path.append(_p)

import numpy as np

B, S, D, H, DK = 2, 2048, 1024, 16, 64
NCORES = 8
GROUPS = 4            # head-groups per batch (cores per batch)
HG = H // GROUPS      # heads per core = 4
F = HG * DK           # features per core = 256
EB = D // 128         # 8 e-blocks
KC = S // 128         # 16 k-chunks
QP = S // 1024        # 2 q-pair-chunks of 1024


def _round_f32r(x: np.ndarray) -> np.ndarray:
    """Round f32 to the fp32r grid (11-bit mantissa): RNE at bit 12."""
    u = np.ascontiguousarray(x, dtype=np.float32).view(np.uint32)
    low = u & np.uint32(0xFFF)
    half = np.uint32(0x800)
    lsb = (u >> np.uint32(12)) & np.uint32(1)
    round_up = (low > half) | ((low == half) & (lsb == 1))
    u = (u & np.uint32(0xFFFFF000)) + (round_up.astype(np.uint32) << np.uint32(12))
    return u.view(np.float32)


def build_nc(reps: int = 1):
    import concourse.bass as bass
    import concourse.tile as tile
    from concourse import bacc, mybir

    F32 = mybir.dt.float32
    F32R = mybir.dt.float32r
    EXP = mybir.ActivationFunctionType.Exp
    MULT = mybir.AluOpType.mult

    nc = bacc.Bacc("TRN2", target_bir_lowering=False, debug=False,
                   num_devices=NCORES)

    xq = nc.dram_tensor("xq", [D, S], F32R, kind="ExternalInput").ap()
    xk = nc.dram_tensor("xk", [D, S], F32R, kind="ExternalInput").ap()
    xv = nc.dram_tensor("xv", [D, S], F32R, kind="ExternalInput").ap()
    wq = nc.dram_tensor("wq", [D, F], F32R, kind="ExternalInput").ap()
    wk = nc.dram_tensor("wk", [D, F], F32R, kind="ExternalInput").ap()
    wv = nc.dram_tensor("wv", [D, F], F32R, kind="ExternalInput").ap()
    wo = nc.dram_tensor("wo", [F, D], F32R, kind="ExternalInput").ap()
    outT = nc.dram_tensor("outT", [D, S], F32, kind="ExternalOutput").ap()
    # scratch for the softmax-denominator partition-broadcast bounce

    with tile.TileContext(nc) as tc:
        with (
            tc.tile_pool(name="persist", bufs=1) as pp,
            tc.tile_pool(name="woP", bufs=1) as wop,
            tc.tile_pool(name="xs", bufs=2) as xs,
            tc.tile_pool(name="xvp", bufs=1) as xvp,
            tc.tile_pool(name="ws", bufs=2) as ws,
            tc.tile_pool(name="pt", bufs=3) as ptp,
            tc.tile_pool(name="sm", bufs=2) as smp,
            tc.tile_pool(name="ob", bufs=2) as obp,
            tc.tile_pool(name="ps", bufs=2, space="PSUM") as psp,
        ):
            KT = pp.tile([128, 2, S], F32R, tag="kt")    # heads 2fb,2fb+1 on rows
            QT = pp.tile([128, 2, S], F32R, tag="qt")
            Vp = pp.tile([128, KC, HG * (DK + 1)], F32R, tag="vp")  # V'+ones per kc
            CT = pp.tile([128, 2, S], F32R, tag="ct")    # normalized ctx^T
            WO = wop.tile([128, 2, D], F32R, tag="wo")
            ones = nc.const_aps.scalar_like(1.0, Vp[:])
            ones_row = wop.tile([1, 64], F32R, tag="onesrow")
            nc.vector.tensor_copy(
                ones_row[:], nc.const_aps.tensor(1.0, (1, 1), F32)
                .to_broadcast((1, 64)))

            def psum(tag, name):
                # two 2-bank tags, bufs=2 each -> 8 banks total
                return psp.tile([128, 1024], F32, tag=tag, name=name)

            def proj_qT(w_sb, x_t, ps_pair, eb):
                for fb in range(2):
                    for qh in range(2):
                        nc.tensor.matmul(
                            ps_pair[fb][:, qh * 512:(qh + 1) * 512],
                            w_sb[:, eb, fb * 128:(fb + 1) * 128],
                            x_t[:, qh * 512:(qh + 1) * 512],
                            start=(eb == 0), stop=(eb == EB - 1))

            for _rep in range(reps):
                # ---------------- V natural (streamed-in resident xv) ----------
                wv_sb = ws.tile([128, EB, F], F32R, tag="w", name="wv_sb")
                nc.sync.dma_start(wv_sb[:], wv.rearrange("(c p) f -> p c f", p=128))
                xv_sb = xvp.tile([128, EB, S], F32R, tag="xv")
                for eb in range(EB):
                    nc.sync.dma_start(xv_sb[:, eb, :], xv[eb * 128:(eb + 1) * 128, :])
                for rnd in range(2):
                    kcs = list(range(rnd * 8, rnd * 8 + 8))
                    vps = {}
                    for j in range(4):  # tile j holds kc pair (2j, 2j+1) of round
                        t = psum(("kq", "ctx")[j % 2], f"vps{rnd}_{j}")
                        vps[kcs[2 * j]] = t[:, 0:256]        # bank 0
                        vps[kcs[2 * j + 1]] = t[:, 512:768]  # bank 1
                    for eb in range(EB):
                        for kc in kcs:
                            nc.tensor.matmul(
                                vps[kc],
                                xv_sb[:, eb, kc * 128:(kc + 1) * 128],
                                wv_sb[:, eb, :],
                                start=(eb == 0), stop=(eb == EB - 1))
                    for kc in kcs:
                        for h in range(HG):
                            nc.vector.tensor_copy(
                                Vp[:, kc, h * 65:h * 65 + 64],
                                vps[kc][:, h * 64:(h + 1) * 64])
                        nc.vector.tensor_copy(
                            Vp[:, kc, 64::65], ones.to_broadcast((128, HG)))

                # ---------------- K^T (streamed xk) ----------------
                wk_sb = ws.tile([128, EB, F], F32R, tag="w", name="wk_sb")
                nc.sync.dma_start(wk_sb[:], wk.rearrange("(c p) f -> p c f", p=128))
                kps = {(fb, qp): psum(("kq", "ctx")[fb], f"kps{fb}{qp}")
                       for fb in range(2) for qp in range(QP)}
                for eb in range(EB):
                    x_t = xs.tile([128, S], F32R, tag="x", name=f"xk_t{eb}")
                    nc.sync.dma_start(x_t[:], xk[eb * 128:(eb + 1) * 128, :])
                    for qp in range(QP):
                        proj_qT(wk_sb, x_t[:, qp * 1024:(qp + 1) * 1024],
                                [kps[0, qp], kps[1, qp]], eb)
                for fb in range(2):
                    for qp in range(QP):
                        nc.vector.tensor_copy(
                            KT[:, fb, qp * 1024:(qp + 1) * 1024], kps[fb, qp][:])

                # ---------------- Q^T qp0 (streamed half tiles) ----------------
                wq_sb = ws.tile([128, EB, F], F32R, tag="w", name="wq_sb")
                nc.sync.dma_start(wq_sb[:], wq.rearrange("(c p) f -> p c f", p=128))
                qps0 = [psum(("kq", "ctx")[i], f"qps0_{i}") for i in range(2)]
                for eb in range(EB):
                    x_t = xs.tile([128, 1024], F32R, tag="x", name=f"xq0_t{eb}")
                    nc.sync.dma_start(x_t[:], xq[eb * 128:(eb + 1) * 128, 0:1024])
                    proj_qT(wq_sb, x_t, qps0, eb)
                for fb in range(2):
                    nc.vector.tensor_copy(QT[:, fb, 0:1024], qps0[fb][:])

                def proj_q1():
                    qps1 = [psum(("kq", "ctx")[i], f"qps1_{i}") for i in range(2)]
                    for eb in range(EB):
                        x_t = xs.tile([128, 1024], F32R, tag="x", name=f"xq1_t{eb}")
                        nc.sync.dma_start(x_t[:],
                                          xq[eb * 128:(eb + 1) * 128, 1024:2048])
                        proj_qT(wq_sb, x_t, qps1, eb)
                    for fb in range(2):
                        nc.vector.tensor_copy(QT[:, fb, 1024:2048], qps1[fb][:])

                # ============== attention + output projection ==============
                def attention_head(qp, h):
                    fb, ro = divmod(h, 2)
                    ro *= 64
                    qsl = slice(qp * 1024, (qp + 1) * 1024)
                    ctx_ps = psp.tile([65, 1024], F32, tag="ctx",
                                      name=f"ctx{qp}{h}")
                    for kc in range(KC):
                        s_ps = psp.tile([128, 1024], F32, tag="kq",
                                        name=f"s{qp}{h}{kc}")
                        for qh in range(2):
                            nc.tensor.matmul(
                                s_ps[:, qh * 512:(qh + 1) * 512],
                                KT[ro:ro + 64, fb, kc * 128:(kc + 1) * 128],
                                QT[ro:ro + 64, fb, qp * 1024 + qh * 512:][:, :512],
                                start=True, stop=True)
                        p_t = ptp.tile([128, 1024], F32R, tag="p", name="p_t")
                        nc.scalar.activation(p_t[:], s_ps[:], EXP,
                                             scale=1.0 / np.sqrt(DK))
                        for qh in range(2):
                            nc.tensor.matmul(
                                ctx_ps[:, qh * 512:(qh + 1) * 512],
                                Vp[:, kc, h * 65:(h + 1) * 65],
                                p_t[:, qh * 512:(qh + 1) * 512],
                                start=(kc == 0), stop=(kc == KC - 1))
                    # normalize rows 0..63 by row 64 (softmax denominators)
                    rc = smp.tile([1, 1024], F32R, tag="rc", name="rc")
                    with nc.allow_low_precision(reason="f32r recip bcast"):
                        nc.vector.reciprocal(rc[:], ctx_ps[64:65, :])
                    bc_ps = psp.tile([64, 1024], F32, tag="ctx",
                                     name=f"bc{qp}{h}")
                    for qh in range(2):
                        nc.tensor.matmul(
                            bc_ps[:, qh * 512:(qh + 1) * 512], ones_row[:],
                            rc[:, qh * 512:(qh + 1) * 512],
                            start=True, stop=True)
                    bc = smp.tile([64, 1024], F32, tag="bc", name="bc")
                    nc.vector.tensor_copy(bc[:], bc_ps[:])
                    nc.vector.tensor_tensor(
                        CT[ro:ro + 64, fb, qsl], ctx_ps[0:64, :], bc[:], MULT)

                def outproj(qp, ods):
                    qsl = slice(qp * 1024, (qp + 1) * 1024)
                    for od in ods:
                        o_ps = psp.tile([128, 1024], F32, tag="ctx",
                                        name=f"o{qp}{od}")
                        for qh in range(2):
                            for fb in range(2):
                                nc.tensor.matmul(
                                    o_ps[:, qh * 512:(qh + 1) * 512],
                                    WO[:, fb, od * 128:(od + 1) * 128],
                                    CT[:, fb, qp * 1024 + qh * 512:][:, :512],
                                    start=(fb == 0), stop=(fb == 1))
                        o_sb = obp.tile([128, 1024], F32, tag="ob", name="o_sb")
                        nc.vector.tensor_copy(o_sb[:], o_ps[:])
                        nc.sync.dma_start(outT[od * 128:(od + 1) * 128, qsl],
                                          o_sb[:])

                attention_head(0, 0)
                proj_q1()   # Q^T qp1 overlaps attention of (qp0, h0)
                nc.sync.dma_start(WO[:], wo.rearrange("(c p) o -> p c o", p=128))
                for h in range(1, HG):
                    attention_head(0, h)
                # interleave qp0's out-proj with qp1's attention
                attention_head(1, 0)
                outproj(0, (0, 1))
                attention_head(1, 1)
                outproj(0, (2, 3))
                attention_head(1, 2)
                outproj(0, (4, 5))
                attention_head(1, 3)
                outproj(0, (6, 7))
                outproj(1, range(8))
    nc.compile()
    nc.remove_dead_allocations()
    return nc


_CACHE = {}


def _get_nc(reps: int = 1):
    if reps not in _CACHE:
        _CACHE[reps] = build_nc(reps)
    return _CACHE[reps]


def make_in_maps(key, query, value, Wq, Wk, Wv, Wo):
    """Build the 8 per-core input maps (host-side shard + transpose + round)."""
    in_maps = []
    xqT = [_round_f32r(np.ascontiguousarray(query[b].T)) for b in range(B)]
    xkT = [_round_f32r(np.ascontiguousarray(key[b].T)) for b in range(B)]
    xvT = [_round_f32r(np.ascontiguousarray(value[b].T)) for b in range(B)]
    for c in range(NCORES):
        b, g = divmod(c, GROUPS)
        fs = slice(g * F, (g + 1) * F)
        in_maps.append({
            "xq": xqT[b],
            "xk": xkT[b],
            "xv": xvT[b],
            "wq": _round_f32r(np.ascontiguousarray(Wq[fs, :].T)),
            "wk": _round_f32r(np.ascontiguousarray(Wk[fs, :].T)),
            "wv": _round_f32r(np.ascontiguousarray(Wv[fs, :].T)),
            "wo": _round_f32r(np.ascontiguousarray(Wo[:, fs].T)),
        })
    return in_maps


def gather_output(results, bo):
    """results: list of 8 dicts with 'outT' [D, S] partials."""
    out = np.empty((B, S, D), dtype=np.float32)
    for b in range(B):
        acc = np.zeros((D, S), dtype=np.float64)
        for g in range(GROUPS):
            acc += results[b * GROUPS + g]["outT"]
        out[b] = acc.T + bo.astype(np.float64)
    return out


def kernel(key, query, value, Wq, Wk, Wv, Wo, bo):
    from concourse.bass_utils import run_bass_kernel_spmd

    key = np.asarray(key, dtype=np.float32)
    query = np.asarray(query, dtype=np.float32)
    value = np.asarray(value, dtype=np.float32)
    Wq = np.asarray(Wq, dtype=np.float32)
    Wk = np.asarray(Wk, dtype=np.float32)
    Wv = np.asarray(Wv, dtype=np.float32)
    Wo = np.asarray(Wo, dtype=np.float32)
    bo = np.asarray(bo, dtype=np.float32)

    nc = _get_nc()
    in_maps = make_in_maps(key, query, value, Wq, Wk, Wv, Wo)
    res = run_bass_kernel_spmd(nc, in_maps, core_ids=list(range(NCORES)))
    return gather_output(res.results, bo)


# revision 22
# speedup vs baseline: 1.0676x; 1.0228x over previous
"""Multi-head attention (B=2, S=2048, D=1024, H=16, dk=64) on 8 TRN2 NeuronCores.

Sharding: core c -> (batch b = c//4, head-group g = c%4 covering heads
[4g, 4g+4) == feature columns [256g, 256g+256)).  Each core:
  - projects K^T/Q^T (features on partitions) and V (natural layout) for its
    4 heads from the batch's activations (host-pre-transposed),
  - runs softmax attention per head without max-subtraction (scores ~ N(0,1));
    softmax denominators come from an appended ones-column on V,
  - computes the transposed partial output  outT = (Wo_cols.T @ ctx^T)  [1024, 2048].
Host gathers: out[b] = sum_g outT[b,g].T + bo.

All matmul operands use fp32r (TF32-like, 11-bit mantissa) at full PE rate;
accumulation is fp32 in PSUM.  End-to-end rel-err vs fp32 reference ~1e-4.

Schedule: K-proj (streamed) -> Q-proj qp0 -> V-proj (resident) -> Q-proj qp1
-> attention qp0 -> attention qp1 (with qp0's output projection interleaved)
-> output projection qp1.  One PSUM pool with two 2-bank tags shared by all
phases so no pool-boundary serialization; attention overlaps late projections.
"""
import sys

for _p in ("/opt/trn_rl_repo", "/root/.axon_site/_ro/trn_rl_repo"):
    if _p not in sys.path:
        sys.path.append(_p)

import numpy as np

B, S, D, H, DK = 2, 2048, 1024, 16, 64
NCORES = 8
GROUPS = 4            # head-groups per batch (cores per batch)
HG = H // GROUPS      # heads per core = 4
F = HG * DK           # features per core = 256
EB = D // 128         # 8 e-blocks
KC = S // 128         # 16 k-chunks
QP = S // 1024        # 2 q-pair-chunks of 1024


def _round_f32r(x: np.ndarray) -> np.ndarray:
    """Round f32 to the fp32r grid (11-bit mantissa): RNE at bit 12."""
    u = np.ascontiguousarray(x, dtype=np.float32).view(np.uint32)
    low = u & np.uint32(0xFFF)
    half = np.uint32(0x800)
    lsb = (u >> np.uint32(12)) & np.uint32(1)
    round_up = (low > half) | ((low == half) & (lsb == 1))
    u = (u & np.uint32(0xFFFFF000)) + (round_up.astype(np.uint32) << np.uint32(12))
    return u.view(np.float32)


def build_nc(reps: int = 1):
    import concourse.bass as bass
    import concourse.tile as tile
    from concourse import bacc, mybir

    F32 = mybir.dt.float32
    F32R = mybir.dt.float32r
    EXP = mybir.ActivationFunctionType.Exp
    MULT = mybir.AluOpType.mult

    nc = bacc.Bacc("TRN2", target_bir_lowering=False, debug=False,
                   num_devices=NCORES)

    xq = nc.dram_tensor("xq", [D, S], F32R, kind="ExternalInput").ap()
    xk = nc.dram_tensor("xk", [D, S], F32R, kind="ExternalInput").ap()
    xv = nc.dram_tensor("xv", [D, S], F32R, kind="ExternalInput").ap()
    wq = nc.dram_tensor("wq", [D, F], F32R, kind="ExternalInput").ap()
    wk = nc.dram_tensor("wk", [D, F], F32R, kind="ExternalInput").ap()
    wv = nc.dram_tensor("wv", [D, F], F32R, kind="ExternalInput").ap()
    wo = nc.dram_tensor("wo", [F, D], F32R, kind="ExternalInput").ap()
    outT = nc.dram_tensor("outT", [D, S], F32, kind="ExternalOutput").ap()
    # scratch for the softmax-denominator partition-broadcast bounce

    with tile.TileContext(nc) as tc:
        with (
            tc.tile_pool(name="persist", bufs=1) as pp,
            tc.tile_pool(name="woP", bufs=1) as wop,
            tc.tile_pool(name="xs", bufs=2) as xs,
            tc.tile_pool(name="xvp", bufs=1) as xvp,
            tc.tile_pool(name="ws", bufs=2) as ws,
            tc.tile_pool(name="pt", bufs=3) as ptp,
            tc.tile_pool(name="sm", bufs=2) as smp,
            tc.tile_pool(name="ob", bufs=2) as obp,
            tc.tile_pool(name="ps", bufs=2, space="PSUM") as psp,
        ):
            KT = pp.tile([128, 2, S], F32R, tag="kt")    # heads 2fb,2fb+1 on rows
            QT = pp.tile([128, 2, S], F32R, tag="qt")
            Vp = pp.tile([128, KC, HG * (DK + 1)], F32R, tag="vp")  # V'+ones per kc
            CT = pp.tile([128, 2, S], F32R, tag="ct")    # normalized ctx^T
            WO = wop.tile([128, 2, D], F32R, tag="wo")
            ones = nc.const_aps.scalar_like(1.0, Vp[:])
            ones_row = wop.tile([1, 64], F32R, tag="onesrow")
            nc.vector.tensor_copy(
                ones_row[:], nc.const_aps.tensor(1.0, (1, 1), F32)
                .to_broadcast((1, 64)))

            def psum(tag, name):
                # two 2-bank tags, bufs=2 each -> 8 banks total
                return psp.tile([128, 1024], F32, tag=tag, name=name)

            def proj_qT(w_sb, x_t, ps_pair, eb):
                for fb in range(2):
                    for qh in range(2):
                        nc.tensor.matmul(
                            ps_pair[fb][:, qh * 512:(qh + 1) * 512],
                            w_sb[:, eb, fb * 128:(fb + 1) * 128],
                            x_t[:, qh * 512:(qh + 1) * 512],
                            start=(eb == 0), stop=(eb == EB - 1))

            for _rep in range(reps):
                # ---------------- V natural (streamed-in resident xv) ----------
                wv_sb = ws.tile([128, EB, F], F32R, tag="w", name="wv_sb")
                nc.sync.dma_start(wv_sb[:], wv.rearrange("(c p) f -> p c f", p=128))
                xv_sb = xvp.tile([128, EB, S], F32R, tag="xv")
                for eb in range(EB):
                    nc.sync.dma_start(xv_sb[:, eb, :], xv[eb * 128:(eb + 1) * 128, :])
                for rnd in range(2):
                    kcs = list(range(rnd * 8, rnd * 8 + 8))
                    vps = {}
                    for j in range(4):  # tile j holds kc pair (2j, 2j+1) of round
                        t = psum(("kq", "ctx")[j % 2], f"vps{rnd}_{j}")
                        vps[kcs[2 * j]] = t[:, 0:256]        # bank 0
                        vps[kcs[2 * j + 1]] = t[:, 512:768]  # bank 1
                    for eb in range(EB):
                        for kc in kcs:
                            nc.tensor.matmul(
                                vps[kc],
                                xv_sb[:, eb, kc * 128:(kc + 1) * 128],
                                wv_sb[:, eb, :],
                                start=(eb == 0), stop=(eb == EB - 1))
                    for kc in kcs:
                        for h in range(HG):
                            nc.vector.tensor_copy(
                                Vp[:, kc, h * 65:h * 65 + 64],
                                vps[kc][:, h * 64:(h + 1) * 64])
                        nc.vector.tensor_copy(
                            Vp[:, kc, 64::65], ones.to_broadcast((128, HG)))

                # ---------------- K^T (streamed xk) ----------------
                wk_sb = ws.tile([128, EB, F], F32R, tag="w", name="wk_sb")
                nc.sync.dma_start(wk_sb[:], wk.rearrange("(c p) f -> p c f", p=128))
                kps = {(fb, qp): psum(("kq", "ctx")[fb], f"kps{fb}{qp}")
                       for fb in range(2) for qp in range(QP)}
                for eb in range(EB):
                    x_t = xs.tile([128, S], F32R, tag="x", name=f"xk_t{eb}")
                    nc.sync.dma_start(x_t[:], xk[eb * 128:(eb + 1) * 128, :])
                    for qp in range(QP):
                        proj_qT(wk_sb, x_t[:, qp * 1024:(qp + 1) * 1024],
                                [kps[0, qp], kps[1, qp]], eb)
                for fb in range(2):
                    for qp in range(QP):
                        nc.vector.tensor_copy(
                            KT[:, fb, qp * 1024:(qp + 1) * 1024], kps[fb, qp][:])

                # ---------------- Q^T qp0 (streamed half tiles) ----------------
                wq_sb = ws.tile([128, EB, F], F32R, tag="w", name="wq_sb")
                nc.sync.dma_start(wq_sb[:], wq.rearrange("(c p) f -> p c f", p=128))
                qps0 = [psum(("kq", "ctx")[i], f"qps0_{i}") for i in range(2)]
                for eb in range(EB):
                    x_t = xs.tile([128, 1024], F32R, tag="x", name=f"xq0_t{eb}")
                    nc.sync.dma_start(x_t[:], xq[eb * 128:(eb + 1) * 128, 0:1024])
                    proj_qT(wq_sb, x_t, qps0, eb)
                for fb in range(2):
                    nc.vector.tensor_copy(QT[:, fb, 0:1024], qps0[fb][:])

                def proj_q1():
                    qps1 = [psum(("kq", "ctx")[i], f"qps1_{i}") for i in range(2)]
                    for eb in range(EB):
                        x_t = xs.tile([128, 1024], F32R, tag="x", name=f"xq1_t{eb}")
                        nc.sync.dma_start(x_t[:],
                                          xq[eb * 128:(eb + 1) * 128, 1024:2048])
                        proj_qT(wq_sb, x_t, qps1, eb)
                    for fb in range(2):
                        nc.vector.tensor_copy(QT[:, fb, 1024:2048], qps1[fb][:])

                # ============== attention + output projection ==============
                def attention_head(qp, h):
                    fb, ro = divmod(h, 2)
                    ro *= 64
                    qsl = slice(qp * 1024, (qp + 1) * 1024)
                    ctx_ps = psp.tile([65, 1024], F32, tag="ctx",
                                      name=f"ctx{qp}{h}")
                    for kc in range(KC):
                        s_ps = psp.tile([128, 1024], F32, tag="kq",
                                        name=f"s{qp}{h}{kc}")
                        for qh in range(2):
                            nc.tensor.matmul(
                                s_ps[:, qh * 512:(qh + 1) * 512],
                                KT[ro:ro + 64, fb, kc * 128:(kc + 1) * 128],
                                QT[ro:ro + 64, fb, qp * 1024 + qh * 512:][:, :512],
                                start=True, stop=True)
                        p_t = ptp.tile([128, 1024], F32R, tag="p", name="p_t")
                        nc.scalar.activation(p_t[:], s_ps[:], EXP,
                                             scale=1.0 / np.sqrt(DK))
                        for qh in range(2):
                            nc.tensor.matmul(
                                ctx_ps[:, qh * 512:(qh + 1) * 512],
                                Vp[:, kc, h * 65:(h + 1) * 65],
                                p_t[:, qh * 512:(qh + 1) * 512],
                                start=(kc == 0), stop=(kc == KC - 1))
                    # normalize rows 0..63 by row 64 (softmax denominators)
                    rc = smp.tile([1, 1024], F32R, tag="rc", name="rc")
                    with nc.allow_low_precision(reason="f32r recip bcast"):
                        nc.vector.reciprocal(rc[:], ctx_ps[64:65, :])
                    bc_ps = psp.tile([64, 1024], F32, tag="ctx",
                                     name=f"bc{qp}{h}")
                    for qh in range(2):
                        nc.tensor.matmul(
                            bc_ps[:, qh * 512:(qh + 1) * 512], ones_row[:],
                            rc[:, qh * 512:(qh + 1) * 512],
                            start=True, stop=True)
                    bc = smp.tile([64, 1024], F32, tag="bc", name="bc")
                    nc.vector.tensor_copy(bc[:], bc_ps[:])
                    nc.vector.tensor_tensor(
                        CT[ro:ro + 64, fb, qsl], ctx_ps[0:64, :], bc[:], MULT)

                def outproj(qp, ods):
                    qsl = slice(qp * 1024, (qp + 1) * 1024)
                    for od in ods:
                        o_ps = psp.tile([128, 1024], F32, tag="ctx",
                                        name=f"o{qp}{od}")
                        for qh in range(2):
                            for fb in range(2):
                                nc.tensor.matmul(
                                    o_ps[:, qh * 512:(qh + 1) * 512],
                                    WO[:, fb, od * 128:(od + 1) * 128],
                                    CT[:, fb, qp * 1024 + qh * 512:][:, :512],
                                    start=(fb == 0), stop=(fb == 1))
                        o_sb = obp.tile([128, 1024], F32, tag="ob", name="o_sb")
                        nc.vector.tensor_copy(o_sb[:], o_ps[:])
                        nc.sync.dma_start(outT[od * 128:(od + 1) * 128, qsl],
                                          o_sb[:])

                attention_head(0, 0)
                proj_q1()   # Q^T qp1 overlaps attention of (qp0, h0)
                nc.sync.dma_start(WO[:], wo.rearrange("(c p) o -> p c o", p=128))
                for h in range(1, HG):
                    attention_head(0, h)
                # interleave qp0's out-proj with qp1's attention
                attention_head(1, 0)
                outproj(0, (0, 1))
                attention_head(1, 1)
                outproj(0, (2, 3))
                attention_head(1, 2)
                outproj(0, (4, 5))
                attention_head(1, 3)
                outproj(0, (6, 7))
                outproj(1, range(8))
    nc.compile()
    nc.remove_dead_allocations()
    return nc


_CACHE = {}


def _get_nc(reps: int = 1):
    if reps not in _CACHE:
        _CACHE[reps] = build_nc(reps)
    return _CACHE[reps]


def make_in_maps(key, query, value, Wq, Wk, Wv, Wo):
    """Build the 8 per-core input maps (host-side shard + transpose + round)."""
    in_maps = []
    xqT = [_round_f32r(np.ascontiguousarray(query[b].T)) for b in range(B)]
    xkT = [_round_f32r(np.ascontiguousarray(key[b].T)) for b in range(B)]
    xvT = [_round_f32r(np.ascontiguousarray(value[b].T)) for b in range(B)]
    for c in range(NCORES):
        b, g = divmod(c, GROUPS)
        fs = slice(g * F, (g + 1) * F)
        in_maps.append({
            "xq": xqT[b],
            "xk": xkT[b],
            "xv": xvT[b],
            "wq": _round_f32r(np.ascontiguousarray(Wq[fs, :].T)),
            "wk": _round_f32r(np.ascontiguousarray(Wk[fs, :].T)),
            "wv": _round_f32r(np.ascontiguousarray(Wv[fs, :].T)),
            "wo": _round_f32r(np.ascontiguousarray(Wo[:, fs].T)),
        })
    return in_maps


def gather_output(results, bo):
    """results: list of 8 dicts with 'outT' [D, S] partials."""
    out = np.empty((B, S, D), dtype=np.float32)
    for b in range(B):
        acc = np.zeros((D, S), dtype=np.float64)
        for g in range(GROUPS):
            acc += results[b * GROUPS + g]["outT"]
        out[b] = acc.T + bo.astype(np.float64)
    return out


def kernel(key, query, value, Wq, Wk, Wv, Wo, bo):
    from concourse.bass_utils import run_bass_kernel_spmd

    key = np.asarray(key, dtype=np.float32)
    query = np.asarray(query, dtype=np.float32)
    value = np.asarray(value, dtype=np.float32)
    Wq = np.asarray(Wq, dtype=np.float32)
    Wk = np.asarray(Wk, dtype=np.float32)
    Wv = np.asarray(Wv, dtype=np.float32)
    Wo = np.asarray(Wo, dtype=np.float32)
    bo = np.asarray(bo, dtype=np.float32)

    nc = _get_nc()
    in_maps = make_in_maps(key, query, value, Wq, Wk, Wv, Wo)
    res = run_bass_kernel_spmd(nc, in_maps, core_ids=list(range(NCORES)))
    return gather_output(res.results, bo)
